# revision 42
# baseline (speedup 1.0000x reference)
"""Two-layer GCN encoder on 8 Trainium2 NeuronCores (Bass/Tile).

Strategy (edge-parallel by destination range, v5):
  - Host precomputes degrees/normalizations and pre-gathers the layer-1
    messages xs[src] = (dis*x)[src] per edge slot; self-loops are folded in
    as extra host-side edges, so layer 1 needs no device-side gather or
    identity matmuls at all.
  - Core k owns dst range [6400k, 6400(k+1)); layer-1 edges grouped by dst
    tile (128 nodes) and dst QUARTER-tile (32 nodes): the one-hot is built at
    32-node width (2x less DVE work than 64, speeding zt production); segment
    matmuls write the 4 quarter-ranges of a shared PSUM accumulator via PE
    tile positioning (tile_position=(0, 32q)).
  - Layer-2 edges are split by (src-owner collective chunk x src parity x dst
    half), each (tile, class) segment padded to a multiple of 128 so every
    128-edge chunk is single-class.
  - Segment sums run on the tensor engine with the one-hot(dst) chunk as the
    stationary lhsT and the messages streaming as rhs, accumulating
    node-major results in PSUM.
  - GCN algebra: zt = dis^2*relu((A_hat@xs)@W1 + invdis*b1)@W2,
    out = dis*(A_hat@zt) + b2, with self loops as extra terms/edges.
  - zt (64 cols) is AllGathered in 4 chunks (10/15/15/10 tiles, tuned so
    per-class edge counts sit just under the 128-slot padding boundaries)
    issued mid-pass-1; layer-2 gathers for chunk c run while chunk c+1's
    collective is in flight, so only the last chunk's gathers are exposed.
  - The two tail chunks ship zt as fp8(e4m3) (column-duplicated 128B rows, so
    pair indices are unchanged): the 64B gather payload hits the 7ns
    descriptor-time floor, shrinking the exposed gather tail; ~50% of
    messages at fp8 measures 6.6e-3 rel err vs the 2e-2 gate.
  - Gathers run as 10-tile units (2 matmul groups each) to amortize the ~1us
    fixed SWDGE desc-gen overhead on the Pool engine, reading 128B (fp16) /
    64B (fp8) payloads from 256B-strided pair-packed rows (even/odd src
    classes gather from +0B/+128B or +0B/+128B-row base offsets).
"""
import sys

sys.path.insert(0, "/opt/trn_rl_repo")

import numpy as np

from concourse import bacc, mybir, tile
from concourse import library_config

P = 128
H = 64                        # layer-2 one-hot node-group width (half tile)
H1 = 32                       # layer-1 one-hot width (quarter tile)
NE1 = P // H1                 # 4 quarters per tile
NCORES = 8
N_NODES = 50000
RANGE = 6400                  # nodes per core (50 tiles of 128)
NT = RANGE // P               # 50 node tiles per core
V = NCORES * RANGE            # 51200 padded table rows
F2 = 64                       # zt / output cols
FX = 5                        # raw x feature count
MW = 6                        # layer-1 message row width (fp16), 5 used
GT = 5                        # tiles per layer-2 matmul group
HG = 25                       # tiles per merged gather (half of NT)
ZG = 5                        # tiles per ztown write group
PAD_DST = 9999                # one-hot miss value for padded edge slots
CB = (10, 25, 40)             # collective chunk tile boundaries
CT = (0,) + CB + (NT,)        # chunk tile edges -> sizes (10, 15, 15, 10)
NCH = len(CT) - 1             # collective chunks
NCL = 4 * NCH                 # slot classes (chunk x src parity x dst half)
# each collective reads ztown tiles [CT[c], CT[c+1]) — those writes are
# flushed in groups of ZG tiles, so the boundaries must be ZG-aligned or the
# collective races ahead of the last write
assert all(b % ZG == 0 for b in CB)

f16 = mybir.dt.float16
f32 = mybir.dt.float32
f8 = mybir.dt.float8e4
i16 = mybir.dt.int16
FP8C = NCH - 2                # chunks >= FP8C use fp8 messages (tail chunks)

_prog_cache = {}


def dma_gather_raw(gp, out_ap, in_ap, idxs_ap, num_idxs, elem_size, elem_step):
    """bass.dma_gather minus the 256B elem_size restriction (that assert is
    only required by the firmware's transpose path; the non-transpose Q7
    desc-gen supports any payload size with a 256B-multiple row stride)."""
    assert idxs_ap.dtype == mybir.dt.int16
    assert in_ap.dtype == out_ap.dtype
    assert in_ap.ap[0][0] == elem_step
    stride_bytes = elem_step * mybir.dt.size(in_ap.dtype)
    assert stride_bytes % 256 == 0
    stride_bytes_256 = stride_bytes // 256
    assert stride_bytes_256 < 256
    assert in_ap.ap[-1][1] == out_ap.ap[-1][1] == elem_size
    assert out_ap.ap[0][1] * out_ap.ap[1][1] == ((num_idxs + 127) // 128) * 128

    _in_ap = gp.lower_ap_dma(in_ap, for_custom_bir_dma=True)
    _idxs_ap = gp.lower_ap(idxs_ap)
    _out_ap = gp.lower_ap(out_ap)
    return gp.add_instruction(
        mybir.InstDMAGatherAnt(
            name=gp.bass.get_next_instruction_name(),
            ins=[
                *_in_ap,
                _idxs_ap,
                gp.lower_val_access(gp.to_reg(num_idxs)),
            ],
            outs=[_out_ap],
            transpose=False,
            num_idxs=num_idxs,
            elem_size=elem_size,
            stride_bytes_256=stride_bytes_256,
            gen_mode=0,
            single_packet=False,
            queue_num=0,
            sbuf_tokens_per_rank=0,
            sbuf_free_dim_per_rank=0,
            sbuf_free_dim_pad_per_rank=0,
            sbuf_byte_offset=0,
        )
    )


def build_program(cpt1o, *cpts):
    """cpt1o: layer-1 chunks per (tile, eighth); cpts: NCL layer-2 chunk
    counts per (collective chunk, src parity, dst half) class."""
    assert len(cpts) == NCL
    cpt = sum(cpts)
    cpt1 = NE1 * cpt1o
    # per-tile column base of each class
    cbase = [0]
    for c in cpts:
        cbase.append(cbase[-1] + c)
    NGRP = NT // GT
    rows_c = [P * (CT[c + 1] - CT[c]) for c in range(NCH)]  # per-core rows

    nc = bacc.Bacc("TRN2", target_bir_lowering=False, debug=False,
                   num_devices=NCORES)

    msgs1 = nc.declare_dram_parameter("msgs1", [P, NT * cpt1, MW], f16, isOutput=False)
    dst_l1_in = nc.declare_dram_parameter("dst_l1", [P, NT * cpt1], f16, isOutput=False)
    dst_rel = nc.declare_dram_parameter("dst_rel", [P, NT * cpt], f16, isOutput=False)
    iota_in = nc.declare_dram_parameter("iota_in", [P, H * cpt], f16, isOutput=False)
    # one idx table per (collective chunk, parity) gather stream; its per-tile
    # layout is [dst-half-0 chunks | dst-half-1 chunks]
    scall = [(cpts[4 * c + 2 * r], cpts[4 * c + 2 * r + 1])
             for c in range(NCH) for r in range(2)]
    idx_in = [nc.declare_dram_parameter(
        f"idx{s}", [P, NT * (s0 + s1) * 8], i16, isOutput=False)
        for s, (s0, s1) in enumerate(scall)]
    w1_in = nc.declare_dram_parameter("w1", [FX, P], f16, isOutput=False)
    b1_in = nc.declare_dram_parameter("b1row", [1, P], f16, isOutput=False)
    w2_in = nc.declare_dram_parameter("w2", [P, F2], f16, isOutput=False)
    b2bc_in = nc.declare_dram_parameter("b2bc", [P, F2], f32, isOutput=False)
    invdis_in = nc.declare_dram_parameter("invdis", [1, RANGE], f16, isOutput=False)
    dis_in = nc.declare_dram_parameter("dis_cols", [P, NT], f32, isOutput=False)
    dis2_in = nc.declare_dram_parameter("dis2_cols", [P, NT], f32, isOutput=False)
    id32_in = nc.declare_dram_parameter("ident32", [P, P], f32, isOutput=False)
    out_ext = nc.declare_dram_parameter("out", [RANGE, F2], f32, isOutput=True)

    ztown = nc.dram_tensor("ztown", [RANGE, F2], f16)
    ztg = [nc.dram_tensor(f"ztg{c}", [NCORES * rows_c[c], F2], f16,
                          addr_space="Shared") for c in range(FP8C)]
    # tail chunks ship fp8: each 128B row holds [fp8 zt | fp8 zt dup]; the
    # pair view is 256B so the same even/odd pair indices work, and the 64B
    # gather payload hits the 7ns descriptor-time floor
    ztown8 = [nc.dram_tensor(f"ztown8_{c}", [P * (CT[c + 1] - CT[c]), 2 * F2],
                             f8) for c in range(FP8C, NCH)]
    ztg8 = [nc.dram_tensor(f"ztg8_{c}", [NCORES * rows_c[c], 2 * F2], f8,
                           addr_space="Shared") for c in range(FP8C, NCH)]

    rg = [list(range(NCORES))]
    mlp = library_config.mlp

    with tile.TileContext(nc) as tc:
        with (
            tc.tile_pool(name="const", bufs=1) as const,
            tc.tile_pool(name="ohp", bufs=3) as ohp,
            tc.tile_pool(name="ohp2", bufs=3) as ohp2,
            tc.tile_pool(name="msg2p", bufs=3) as msg2p,
            tc.tile_pool(name="smallp", bufs=8) as smallp,
            tc.tile_pool(name="outp", bufs=4) as outp,
            tc.tile_pool(name="idxp", bufs=2) as idxp,
        ):
            nc.gpsimd.load_library(mlp)

            # ------------- constants / inputs (oh deps first) -------------
            dst_l1 = const.tile([P, NT * cpt1], f16)
            nc.sync.dma_start(out=dst_l1[:], in_=dst_l1_in[:])
            iota16 = const.tile([P, H * cpt], f16)
            nc.sync.dma_start(out=iota16[:], in_=iota_in[:])
            # split the msgs1 load so tile 0's messages land before the
            # monolithic 2.4MB copy would — the first chunk gates the whole
            # pass-1 pipeline (and thus the first collective's start)
            msgs1_sb = const.tile([P, NT * cpt1, MW], f16)
            mc0 = CT[1] * cpt1
            mcA = 3 * cpt1
            nc.sync.dma_start(out=msgs1_sb[:, 0:mcA, :],
                              in_=msgs1.ap()[:, 0:mcA, :])
            nc.sync.dma_start(out=msgs1_sb[:, mcA:mc0, :],
                              in_=msgs1.ap()[:, mcA:mc0, :])
            # small tables tile-0's chain needs, ahead of the bulk copies
            w1_sb = const.tile([FX, P], f16)
            nc.sync.dma_start(out=w1_sb[:], in_=w1_in[:])
            b1row = const.tile([1, P], f16)
            nc.sync.dma_start(out=b1row[:], in_=b1_in[:])
            w2_sb = const.tile([P, F2], f16)
            nc.sync.dma_start(out=w2_sb[:], in_=w2_in[:])
            invdis_sb = const.tile([1, RANGE], f16)
            nc.sync.dma_start(out=invdis_sb[:], in_=invdis_in[:])
            dis2_cols = const.tile([P, NT], f32)
            nc.sync.dma_start(out=dis2_cols[:], in_=dis2_in[:])
            nc.sync.dma_start(out=msgs1_sb[:, mc0:NT * cpt1, :],
                              in_=msgs1.ap()[:, mc0:NT * cpt1, :])
            dst16 = const.tile([P, NT * cpt], f16)
            nc.sync.dma_start(out=dst16[:], in_=dst_rel[:])
            b2bc = const.tile([P, F2], f32)
            nc.sync.dma_start(out=b2bc[:], in_=b2bc_in[:])
            dis_cols = const.tile([P, NT], f32)
            nc.sync.dma_start(out=dis_cols[:], in_=dis_in[:])
            ident32 = const.tile([P, P], f32)
            nc.sync.dma_start(out=ident32[:], in_=id32_in[:])

            b2g = const.tile([P, GT, F2], f32)
            for j in range(GT):
                nc.vector.tensor_copy(out=b2g[:, j, :], in_=b2bc[:])
            ztf16 = const.tile([P, NT, F2], f16)
            zt8 = const.tile([P, NT - CT[FP8C], 2 * F2], f8)
            pa0 = const.tile([P, NT, F2], f16)
            pa1 = const.tile([P, NT, F2], f16)
            qa = pa1 if (NCH - 2) % 2 == 0 else pa0  # src_pa is dead post-add

            iov = iota16[:].rearrange("p (n c) -> p n c", c=cpt)

            def issue_coll(c):
                if c >= FP8C:
                    nc.gpsimd.collective_compute(
                        "AllGather", mybir.AluOpType.bypass, replica_groups=rg,
                        ins=[ztown8[c - FP8C][:]], outs=[ztg8[c - FP8C][:]],
                    )
                else:
                    nc.gpsimd.collective_compute(
                        "AllGather", mybir.AluOpType.bypass, replica_groups=rg,
                        ins=[ztown[CT[c] * P:CT[c + 1] * P, :]],
                        outs=[ztg[c][:]],
                    )

            # ---------------- pass 1: layer 1 -> zt ----------------
            with (
                tc.tile_pool(name="ps_ga", bufs=3, space="PSUM") as ps_ga,
                tc.tile_pool(name="ps_h1", bufs=3, space="PSUM") as ps_h1,
                tc.tile_pool(name="ps_zt", bufs=2, space="PSUM") as ps_zt,
            ):
                for t in range(NT):
                    # 16-wide one-hot across the tile's 8 eighths, one DVE op
                    oh = ohp.tile([P, H1 * cpt1], f16, tag="oh")
                    ohv = oh[:].rearrange("p (n c) -> p n c", c=cpt1)
                    nc.vector.tensor_tensor(
                        out=ohv[:],
                        in0=dst_l1[:, None, t * cpt1:(t + 1) * cpt1
                                   ].broadcast_to([P, H1, cpt1]),
                        in1=iov[:, 0:H1, 0:cpt1],
                        op=mybir.AluOpType.is_equal,
                    )
                    # feature-major aggregate: ga[f, node] directly (no
                    # transpose stage) — lhsT is the 5-wide message column,
                    # the one-hot streams as rhs
                    ga = ps_ga.tile([FX, P], f32, tag="ga")
                    for o in range(NE1):
                        for k in range(cpt1o):
                            i = o * cpt1o + k
                            nc.tensor.matmul(
                                out=ga[0:FX, o * H1:(o + 1) * H1],
                                lhsT=msgs1_sb[:, t * cpt1 + i, 0:FX],
                                rhs=ohv[:, :, i],
                                start=(k == 0), stop=(k == cpt1o - 1),
                            )
                    s1t = smallp.tile([FX, P], f16, tag="s1t")
                    nc.scalar.copy(out=s1t[:], in_=ga[:])
                    h1p = ps_h1.tile([P, P], f32, tag="h1")
                    nc.tensor.matmul(out=h1p[:], lhsT=w1_sb[:], rhs=s1t[:],
                                     start=True, stop=False)
                    nc.tensor.matmul(out=h1p[:], lhsT=b1row[:],
                                     rhs=invdis_sb[:, t * P:(t + 1) * P],
                                     start=False, stop=True)
                    h1r = smallp.tile([P, P], f16, tag="h1r")
                    nc.scalar.activation(out=h1r[:], in_=h1p[:],
                                         func=mybir.ActivationFunctionType.Relu)
                    ztp = ps_zt.tile([P, F2], f32, tag="ztp")
                    nc.tensor.matmul(out=ztp[:], lhsT=h1r[:], rhs=w2_sb[:],
                                     start=True, stop=True)
                    nc.scalar.activation(out=ztf16[:, t, :], in_=ztp[:],
                                         func=mybir.ActivationFunctionType.Copy,
                                         scale=dis2_cols[:, t:t + 1])
                    if t >= CT[FP8C]:
                        tl8 = t - CT[FP8C]
                        nc.scalar.copy(out=zt8[:, tl8, 0:F2],
                                       in_=ztf16[:, t, :])
                        nc.scalar.copy(out=zt8[:, tl8, F2:2 * F2],
                                       in_=ztf16[:, t, :])
                    if t % ZG == ZG - 1:
                        g = t // ZG
                        if t >= CT[FP8C]:
                            c8 = np.searchsorted(np.array(CT), t, "right") - 1
                            b8 = g * ZG - CT[c8]
                            nc.sync.dma_start(
                                out=ztown8[c8 - FP8C].ap()[
                                    b8 * P:(t + 1 - CT[c8]) * P, :].rearrange(
                                    "(t p) f -> p t f", p=P),
                                in_=zt8[:, g * ZG - CT[FP8C]:t + 1 - CT[FP8C],
                                        :],
                            )
                        else:
                            nc.sync.dma_start(
                                out=ztown.ap()[g * ZG * P:(t + 1) * P,
                                               :].rearrange(
                                    "(t p) f -> p t f", p=P),
                                in_=ztf16[:, g * ZG:t + 1, :],
                            )
                    for c in range(NCH - 1):
                        if t == CT[c + 1] - 1:
                            issue_coll(c)

            # pair-packed views: row r of pv[c] holds nodes (2r, 2r+1) as 256B
            pv = [z.ap().rearrange("(r two) f -> r (two f)", two=2) for z in ztg]
            pv8 = [z.ap().rearrange("(r two) f -> r (two f)", two=2)
                   for z in ztg8]
            dsv = dst16[:].rearrange("p (t i) -> p t i", i=cpt)

            with tc.tile_pool(name="ps_g2", bufs=6, space="PSUM") as ps_g2:
                for c in range(NCH):
                    if c == 1:
                        # issued before phase 1's gathers so it dispatches as
                        # soon as its input is ready (Pool SEQ is in-order)
                        issue_coll(NCH - 1)
                    s0, s1 = scall[2 * c]       # even-src: half0, half1 chunks
                    s2, s3 = scall[2 * c + 1]   # odd-src
                    se, so = s0 + s1, s2 + s3
                    cw = se + so
                    c0 = cbase[4 * c]
                    ie = idxp.tile([P, NT * se * 8], i16, tag="idx0")
                    nc.sync.dma_start(out=ie[:], in_=idx_in[2 * c][:])
                    io = idxp.tile([P, NT * so * 8], i16, tag="idx1")
                    nc.sync.dma_start(out=io[:], in_=idx_in[2 * c + 1][:])

                    def finals(g):
                        ts_ = slice(g * GT, (g + 1) * GT)
                        g2g = g2gs.pop(g)
                        if c == 0:
                            nc.vector.tensor_add(out=pa0[:, ts_, :], in0=g2g[:],
                                                 in1=ztf16[:, ts_, :])
                        elif c < NCH - 1:
                            src_pa = pa0 if c % 2 == 1 else pa1
                            dst_pa = pa1 if c % 2 == 1 else pa0
                            nc.vector.tensor_add(out=dst_pa[:, ts_, :],
                                                 in0=g2g[:],
                                                 in1=src_pa[:, ts_, :])
                            if c == NCH - 2:
                                nc.vector.tensor_tensor(
                                    out=qa[:, ts_, :], in0=dst_pa[:, ts_, :],
                                    in1=dis_cols[:, ts_, None].broadcast_to(
                                        [P, GT, F2]),
                                    op=mybir.AluOpType.mult,
                                )
                                nc.vector.tensor_add(out=qa[:, ts_, :],
                                                     in0=qa[:, ts_, :],
                                                     in1=b2g[:])
                        else:
                            outg = outp.tile([P, GT, F2], f32, tag="outg")
                            nc.vector.tensor_tensor(
                                out=outg[:], in0=g2g[:],
                                in1=dis_cols[:, ts_, None].broadcast_to(
                                    [P, GT, F2]),
                                op=mybir.AluOpType.mult,
                            )
                            nc.vector.tensor_add(out=outg[:], in0=outg[:],
                                                 in1=qa[:, ts_, :])
                            nc.sync.dma_start(
                                out=out_ext.ap()[g * GT * P:(g + 1) * GT * P,
                                                 :].rearrange(
                                    "(t p) f -> p t f", p=P),
                                in_=outg[:],
                            )

                    g2gs = {}
                    UG = 2                 # matmul groups per gather unit
                    for u in range(NGRP // UG):
                        if c >= FP8C:
                            msga = msg2p.tile([P, UG * GT * cw, F2], f8,
                                              tag="msg8")
                            dma_gather_raw(
                                nc.gpsimd, msga[:, 0:UG * GT * se, :],
                                pv8[c - FP8C][:, 0:F2],
                                ie[:, u * UG * GT * se * 8:
                                   (u + 1) * UG * GT * se * 8],
                                UG * GT * se * P, F2, 4 * F2)
                            dma_gather_raw(
                                nc.gpsimd, msga[:, UG * GT * se:UG * GT * cw, :],
                                pv8[c - FP8C][:, 2 * F2:3 * F2],
                                io[:, u * UG * GT * so * 8:
                                   (u + 1) * UG * GT * so * 8],
                                UG * GT * so * P, F2, 4 * F2)
                        else:
                            msga = msg2p.tile([P, UG * GT * cw, F2], f16,
                                              tag="msg2")
                            dma_gather_raw(
                                nc.gpsimd, msga[:, 0:UG * GT * se, :],
                                pv[c][:, 0:F2],
                                ie[:, u * UG * GT * se * 8:
                                   (u + 1) * UG * GT * se * 8],
                                UG * GT * se * P, F2, 2 * F2)
                            dma_gather_raw(
                                nc.gpsimd, msga[:, UG * GT * se:UG * GT * cw, :],
                                pv[c][:, F2:2 * F2],
                                io[:, u * UG * GT * so * 8:
                                   (u + 1) * UG * GT * so * 8],
                                UG * GT * so * P, F2, 2 * F2)
                        for gl in range(UG):
                            g = u * UG + gl
                            # group one-hot, 64-wide: oh[p, n, j, i]
                            oh = ohp2.tile([P, H * GT * cw], f16, tag="oh")
                            ohv = oh[:].rearrange("p (n j i) -> p n j i",
                                                  j=GT, i=cw)
                            nc.vector.tensor_tensor(
                                out=ohv[:],
                                in0=dsv[:, None, g * GT:(g + 1) * GT,
                                        c0:c0 + cw].broadcast_to(
                                    [P, H, GT, cw]),
                                in1=iov[:, 0:H, None, 0:cw].broadcast_to(
                                    [P, H, GT, cw]),
                                op=mybir.AluOpType.is_equal,
                            )
                            g2g = ps_g2.tile([P, GT, F2], f32, tag="g2")
                            g2gs[g] = g2g
                            for j in range(GT):
                                jt = gl * GT + j   # tile within gather unit
                                # columns i: [e-h0 | e-h1 | o-h0 | o-h1]
                                for hh in range(2):
                                    cols = ([(k, jt * se + k)
                                             for k in range(s0)]
                                            if hh == 0 else
                                            [(s0 + k, jt * se + s0 + k)
                                             for k in range(s1)])
                                    ocols = ([(se + k,
                                               UG * GT * se + jt * so + k)
                                              for k in range(s2)]
                                             if hh == 0 else
                                             [(se + s2 + k,
                                               UG * GT * se + jt * so + s2 + k)
                                              for k in range(s3)])
                                    chain = cols + ocols
                                    for q, (i, mcol) in enumerate(chain):
                                        nc.tensor.matmul(
                                            out=g2g[hh * H:(hh + 1) * H, j, :],
                                            lhsT=ohv[:, :, j, i],
                                            rhs=msga[:, mcol, :],
                                            start=(q == 0),
                                            stop=(q == len(chain) - 1),
                                        )
                            if g >= 1:
                                finals(g - 1)
                    finals(NGRP - 1)

    nc.compile()
    return nc


def _wrap_idx(a):
    """Index layout for dma_gather: [j%16, j//16] tiled to 128 partitions."""
    lin = a.reshape(-1)
    w = lin.reshape(-1, 16).T
    return np.ascontiguousarray(np.tile(w, (8, 1)))


def make_inputs(x, edge_index, W1, b1, W2, b2):
    x = np.asarray(x, np.float32)
    ei = np.asarray(edge_index)
    src = ei[0].astype(np.int64)
    dst = ei[1].astype(np.int64)
    E = src.shape[0]

    deg = (np.bincount(dst, minlength=N_NODES) + 1.0).astype(np.float32)
    dis = 1.0 / np.sqrt(deg)
    invdis = np.sqrt(deg)
    dis_pad = np.ones(V, np.float32)
    dis_pad[:N_NODES] = dis
    invdis_pad = np.ones(V, np.float32)
    invdis_pad[:N_NODES] = invdis
    xs16_pad = np.zeros((V, FX), np.float16)
    xs16_pad[:N_NODES] = (x * dis[:, None]).astype(np.float16)

    gtile = (dst >> 7).astype(np.int64)               # 0..390
    dhalf = (dst >> 6) & 1                             # dst half-tile
    r = (src % RANGE).astype(np.int64)
    rt = r >> 7                                        # src's owner-local tile
    chunk = np.searchsorted(np.array(CB, np.int64), rt, side="right")
    cls = (2 * chunk + (src & 1)) * 2 + dhalf
    order = np.lexsort((np.arange(E), cls, gtile))
    s_src = src[order]
    s_dst = dst[order]
    s_gt = gtile[order]
    s_cls = cls[order]

    key = s_gt * NCL + s_cls
    counts = np.bincount(key, minlength=400 * NCL)
    starts = np.zeros(400 * NCL + 1, np.int64)
    np.cumsum(counts, out=starts[1:])
    pos = np.arange(E, dtype=np.int64) - starts[key]

    carr = counts.reshape(400, NCL)
    cpts = [max(1, int(np.ceil(carr[:, c].max() / P))) for c in range(NCL)]
    cpt = sum(cpts)
    cbase = np.zeros(NCL + 1, np.int64)
    np.cumsum(cpts, out=cbase[1:])

    tl = s_gt % NT
    col = tl * cpt + cbase[s_cls] + pos // P
    part = pos % P
    core = s_gt // NT

    # ---- layer 1: append self-loop edges, slot by (tile, eighth) ----
    loops = np.arange(N_NODES, dtype=np.int64)
    l_src = np.concatenate([src, loops])
    l_dst = np.concatenate([dst, loops])
    l_gt = (l_dst >> 7).astype(np.int64)
    l_tl = l_gt % NT
    l_core = l_gt // NT
    kth = l_gt * NE1 + ((l_dst >> 5) & (NE1 - 1))
    counts_th = np.bincount(kth, minlength=400 * NE1)
    starts_th = np.zeros(400 * NE1 + 1, np.int64)
    np.cumsum(counts_th, out=starts_th[1:])
    o2 = np.argsort(kth, kind="stable")
    EL = l_src.shape[0]
    pos_th = np.empty(EL, np.int64)
    pos_th[o2] = np.arange(EL, dtype=np.int64) - starts_th[kth[o2]]
    cpt1o = max(1, int(np.ceil(counts_th.max() / P)))
    cpt1 = NE1 * cpt1o
    col1 = l_tl * cpt1 + ((l_dst >> 5) & (NE1 - 1)) * cpt1o + pos_th // P
    part1 = pos_th % P

    # gather table row (pair index) within the chunk's collective output
    s_r = (s_src % RANGE).astype(np.int64)
    s_ch = s_cls >> 2
    ct0 = np.array(CT[:NCH], np.int64) * P
    rows_c = np.array([P * (CT[c + 1] - CT[c]) for c in range(NCH)], np.int64)
    local = s_r - ct0[s_ch]
    idxval = ((s_src // RANGE) * rows_c[s_ch] + local) >> 1

    w1_16 = np.asarray(W1, np.float16)
    b1row = np.asarray(b1, np.float16).reshape(1, P)
    w2_16 = np.asarray(W2, np.float16)
    b2bc = np.tile(np.asarray(b2, np.float32).reshape(1, F2), (P, 1))
    iota = np.tile(np.repeat(np.arange(H, dtype=np.float16), cpt)[None, :],
                   (P, 1))
    ident32 = np.eye(P, dtype=np.float32)

    scall = [(cpts[4 * c + 2 * rr], cpts[4 * c + 2 * rr + 1])
             for c in range(NCH) for rr in range(2)]

    in_maps = []
    for k in range(NCORES):
        m = core == k
        kc, kp = col[m], part[m]
        ml = l_core == k
        kc1, kp1 = col1[ml], part1[ml]

        msgs1_k = np.zeros((P, NT * cpt1, MW), np.float16)
        msgs1_k[kp1, kc1, 0:FX] = xs16_pad[l_src[ml]]
        dst_l1_k = np.full((P, NT * cpt1), PAD_DST, np.float16)
        dst_l1_k[kp1, kc1] = (l_dst[ml] & (H1 - 1)).astype(np.float16)
        dst16_k = np.full((P, NT * cpt), PAD_DST, np.float16)
        dst16_k[kp, kc] = (s_dst[m] & 63).astype(np.float16)

        idx_maps = {}
        for sidx in range(NCL // 2):
            c, rr = sidx // 2, sidx % 2
            cl0 = 4 * c + 2 * rr        # half-0 class id
            s0, s1 = scall[sidx]
            lin = np.zeros(NT * (s0 + s1) * P, np.int16)
            for hh, (cl, sz, off) in enumerate(
                    (((cl0, s0, 0)), (cl0 + 1, s1, s0))):
                mc = m & (s_cls == cl)
                flat = (tl[mc] * (s0 + s1) + off + pos[mc] // P) * P + part[mc]
                lin[flat] = idxval[mc].astype(np.int16)
            idx_maps[f"idx{sidx}"] = _wrap_idx(lin)

        nsl = slice(k * RANGE, (k + 1) * RANGE)
        dis_k = np.ascontiguousarray(
            dis_pad[nsl].reshape(NT, P).T.astype(np.float32))
        dis2_k = np.ascontiguousarray((dis_k * dis_k).astype(np.float32))
        invdis_k = invdis_pad[nsl].reshape(1, RANGE).astype(np.float16)

        in_maps.append({
            "msgs1": msgs1_k, "dst_l1": dst_l1_k, "dst_rel": dst16_k,
            "iota_in": iota,
            **idx_maps,
            "w1": w1_16, "b1row": b1row, "w2": w2_16,
            "b2bc": b2bc, "invdis": invdis_k,
            "dis_cols": dis_k, "dis2_cols": dis2_k,
            "ident32": ident32,
        })
    return (cpt1o,) + tuple(cpts), in_maps


def kernel(x, edge_index, W1, b1, W2, b2):
    from concourse.bass_utils import run_bass_kernel_spmd

    key, in_maps = make_inputs(x, edge_index, W1, b1, W2, b2)
    if key not in _prog_cache:
        _prog_cache[key] = build_program(*key)
    nc = _prog_cache[key]
    res = run_bass_kernel_spmd(nc, in_maps, list(range(NCORES)))
    out = np.concatenate([res.results[k]["out"] for k in range(NCORES)], axis=0)
    return out[:N_NODES]


# revision 50
# speedup vs baseline: 1.0003x; 1.0003x over previous
"""Two-layer GCN encoder on 8 Trainium2 NeuronCores (Bass/Tile).

Strategy (edge-parallel by destination range, v5):
  - Host precomputes degrees/normalizations and pre-gathers the layer-1
    messages xs[src] = (dis*x)[src] per edge slot; self-loops are folded in
    as extra host-side edges, so layer 1 needs no device-side gather or
    identity matmuls at all.
  - Core k owns dst range [6400k, 6400(k+1)); layer-1 edges grouped by dst
    tile (128 nodes) and dst QUARTER-tile (32 nodes): the one-hot is built at
    32-node width (2x less DVE work than 64, speeding zt production); segment
    matmuls write the 4 quarter-ranges of a shared PSUM accumulator via PE
    tile positioning (tile_position=(0, 32q)).
  - Layer-2 edges are split by (src-owner collective chunk x src parity x dst
    half), each (tile, class) segment padded to a multiple of 128 so every
    128-edge chunk is single-class.
  - Segment sums run on the tensor engine with the one-hot(dst) chunk as the
    stationary lhsT and the messages streaming as rhs, accumulating
    node-major results in PSUM.
  - GCN algebra: zt = dis^2*relu((A_hat@xs)@W1 + invdis*b1)@W2,
    out = dis*(A_hat@zt) + b2, with self loops as extra terms/edges.
  - zt (64 cols) is AllGathered in 4 chunks (10/15/15/10 tiles, tuned so
    per-class edge counts sit just under the 128-slot padding boundaries)
    issued mid-pass-1; layer-2 gathers for chunk c run while chunk c+1's
    collective is in flight, so only the last chunk's gathers are exposed.
  - The two tail chunks ship zt as fp8(e4m3) (column-duplicated 128B rows, so
    pair indices are unchanged): the 64B gather payload hits the 7ns
    descriptor-time floor, shrinking the exposed gather tail; ~50% of
    messages at fp8 measures 6.6e-3 rel err vs the 2e-2 gate.
  - Gathers run as 10-tile units (2 matmul groups each) to amortize the ~1us
    fixed SWDGE desc-gen overhead on the Pool engine, reading 128B (fp16) /
    64B (fp8) payloads from 256B-strided pair-packed rows (even/odd src
    classes gather from +0B/+128B or +0B/+128B-row base offsets).
"""
import sys

sys.path.insert(0, "/opt/trn_rl_repo")

import numpy as np

from concourse import bacc, mybir, tile
from concourse import library_config

P = 128
H = 64                        # layer-2 one-hot node-group width (half tile)
H1 = 32                       # layer-1 one-hot width (quarter tile)
NE1 = P // H1                 # 4 quarters per tile
NCORES = 8
N_NODES = 50000
RANGE = 6400                  # nodes per core (50 tiles of 128)
NT = RANGE // P               # 50 node tiles per core
V = NCORES * RANGE            # 51200 padded table rows
F2 = 64                       # zt / output cols
FX = 5                        # raw x feature count
MW = 6                        # layer-1 message row width (fp16), 5 used
GT = 5                        # tiles per layer-2 matmul group
HG = 25                       # tiles per merged gather (half of NT)
ZG = 5                        # tiles per ztown write group
PAD_DST = 9999                # one-hot miss value for padded edge slots
CB = (10, 25, 40)             # collective chunk tile boundaries
CT = (0,) + CB + (NT,)        # chunk tile edges -> sizes (10, 15, 15, 10)
NCH = len(CT) - 1             # collective chunks
NCL = 4 * NCH                 # slot classes (chunk x src parity x dst half)
# each collective reads ztown tiles [CT[c], CT[c+1]) — those writes are
# flushed in groups of ZG tiles, so the boundaries must be ZG-aligned or the
# collective races ahead of the last write
assert all(b % ZG == 0 for b in CB)

f16 = mybir.dt.float16
f32 = mybir.dt.float32
f8 = mybir.dt.float8e4
i16 = mybir.dt.int16
FP8C = NCH - 2                # chunks >= FP8C use fp8 messages (tail chunks)

_prog_cache = {}


def dma_gather_raw(gp, out_ap, in_ap, idxs_ap, num_idxs, elem_size, elem_step):
    """bass.dma_gather minus the 256B elem_size restriction (that assert is
    only required by the firmware's transpose path; the non-transpose Q7
    desc-gen supports any payload size with a 256B-multiple row stride)."""
    assert idxs_ap.dtype == mybir.dt.int16
    assert in_ap.dtype == out_ap.dtype
    assert in_ap.ap[0][0] == elem_step
    stride_bytes = elem_step * mybir.dt.size(in_ap.dtype)
    assert stride_bytes % 256 == 0
    stride_bytes_256 = stride_bytes // 256
    assert stride_bytes_256 < 256
    assert in_ap.ap[-1][1] == out_ap.ap[-1][1] == elem_size
    assert out_ap.ap[0][1] * out_ap.ap[1][1] == ((num_idxs + 127) // 128) * 128

    _in_ap = gp.lower_ap_dma(in_ap, for_custom_bir_dma=True)
    _idxs_ap = gp.lower_ap(idxs_ap)
    _out_ap = gp.lower_ap(out_ap)
    return gp.add_instruction(
        mybir.InstDMAGatherAnt(
            name=gp.bass.get_next_instruction_name(),
            ins=[
                *_in_ap,
                _idxs_ap,
                gp.lower_val_access(gp.to_reg(num_idxs)),
            ],
            outs=[_out_ap],
            transpose=False,
            num_idxs=num_idxs,
            elem_size=elem_size,
            stride_bytes_256=stride_bytes_256,
            gen_mode=0,
            single_packet=False,
            queue_num=0,
            sbuf_tokens_per_rank=0,
            sbuf_free_dim_per_rank=0,
            sbuf_free_dim_pad_per_rank=0,
            sbuf_byte_offset=0,
        )
    )


def build_program(cpt1o, *cpts):
    """cpt1o: layer-1 chunks per (tile, eighth); cpts: NCL layer-2 chunk
    counts per (collective chunk, src parity, dst half) class."""
    assert len(cpts) == NCL
    cpt = sum(cpts)
    cpt1 = NE1 * cpt1o
    # per-tile column base of each class
    cbase = [0]
    for c in cpts:
        cbase.append(cbase[-1] + c)
    NGRP = NT // GT
    rows_c = [P * (CT[c + 1] - CT[c]) for c in range(NCH)]  # per-core rows

    nc = bacc.Bacc("TRN2", target_bir_lowering=False, debug=False,
                   num_devices=NCORES)

    msgs1 = nc.declare_dram_parameter("msgs1", [P, NT * cpt1, MW], f16, isOutput=False)
    dst_l1_in = nc.declare_dram_parameter("dst_l1", [P, NT * cpt1], f16, isOutput=False)
    dst_rel = nc.declare_dram_parameter("dst_rel", [P, NT * cpt], f16, isOutput=False)
    iota_in = nc.declare_dram_parameter("iota_in", [P, H * cpt], f16, isOutput=False)
    # one idx table per (collective chunk, parity) gather stream; its per-tile
    # layout is [dst-half-0 chunks | dst-half-1 chunks]
    scall = [(cpts[4 * c + 2 * r], cpts[4 * c + 2 * r + 1])
             for c in range(NCH) for r in range(2)]
    idx_in = [nc.declare_dram_parameter(
        f"idx{s}", [P, NT * (s0 + s1) * 8], i16, isOutput=False)
        for s, (s0, s1) in enumerate(scall)]
    w1_in = nc.declare_dram_parameter("w1", [FX, P], f16, isOutput=False)
    b1_in = nc.declare_dram_parameter("b1row", [1, P], f16, isOutput=False)
    w2_in = nc.declare_dram_parameter("w2", [P, F2], f16, isOutput=False)
    b2bc_in = nc.declare_dram_parameter("b2bc", [P, F2], f32, isOutput=False)
    invdis_in = nc.declare_dram_parameter("invdis", [1, RANGE], f16, isOutput=False)
    dis_in = nc.declare_dram_parameter("dis_cols", [P, NT], f32, isOutput=False)
    dis2_in = nc.declare_dram_parameter("dis2_cols", [P, NT], f32, isOutput=False)
    id32_in = nc.declare_dram_parameter("ident32", [P, P], f32, isOutput=False)
    out_ext = nc.declare_dram_parameter("out", [RANGE, F2], f32, isOutput=True)

    ztown = nc.dram_tensor("ztown", [RANGE, F2], f16)
    ztg = [nc.dram_tensor(f"ztg{c}", [NCORES * rows_c[c], F2], f16,
                          addr_space="Shared") for c in range(FP8C)]
    # tail chunks ship fp8: each 128B row holds [fp8 zt | fp8 zt dup]; the
    # pair view is 256B so the same even/odd pair indices work, and the 64B
    # gather payload hits the 7ns descriptor-time floor
    ztown8 = [nc.dram_tensor(f"ztown8_{c}", [P * (CT[c + 1] - CT[c]), 2 * F2],
                             f8) for c in range(FP8C, NCH)]
    ztg8 = [nc.dram_tensor(f"ztg8_{c}", [NCORES * rows_c[c], 2 * F2], f8,
                           addr_space="Shared") for c in range(FP8C, NCH)]

    rg = [list(range(NCORES))]
    mlp = library_config.mlp

    with tile.TileContext(nc) as tc:
        with (
            tc.tile_pool(name="const", bufs=1) as const,
            tc.tile_pool(name="ohp", bufs=3) as ohp,
            tc.tile_pool(name="ohp2", bufs=3) as ohp2,
            tc.tile_pool(name="msg2p", bufs=3) as msg2p,
            tc.tile_pool(name="smallp", bufs=12) as smallp,
            tc.tile_pool(name="outp", bufs=6) as outp,
            tc.tile_pool(name="idxp", bufs=2) as idxp,
        ):
            nc.gpsimd.load_library(mlp)

            # ------------- constants / inputs (oh deps first) -------------
            dst_l1 = const.tile([P, NT * cpt1], f16)
            nc.sync.dma_start(out=dst_l1[:], in_=dst_l1_in[:])
            iota16 = const.tile([P, H * cpt], f16)
            nc.sync.dma_start(out=iota16[:], in_=iota_in[:])
            # split the msgs1 load so tile 0's messages land before the
            # monolithic 2.4MB copy would — the first chunk gates the whole
            # pass-1 pipeline (and thus the first collective's start)
            msgs1_sb = const.tile([P, NT * cpt1, MW], f16)
            mc0 = CT[1] * cpt1
            mcA = 3 * cpt1
            nc.sync.dma_start(out=msgs1_sb[:, 0:mcA, :],
                              in_=msgs1.ap()[:, 0:mcA, :])
            nc.sync.dma_start(out=msgs1_sb[:, mcA:mc0, :],
                              in_=msgs1.ap()[:, mcA:mc0, :])
            # small tables tile-0's chain needs, ahead of the bulk copies
            w1_sb = const.tile([FX, P], f16)
            nc.sync.dma_start(out=w1_sb[:], in_=w1_in[:])
            b1row = const.tile([1, P], f16)
            nc.sync.dma_start(out=b1row[:], in_=b1_in[:])
            w2_sb = const.tile([P, F2], f16)
            nc.sync.dma_start(out=w2_sb[:], in_=w2_in[:])
            invdis_sb = const.tile([1, RANGE], f16)
            nc.sync.dma_start(out=invdis_sb[:], in_=invdis_in[:])
            dis2_cols = const.tile([P, NT], f32)
            nc.sync.dma_start(out=dis2_cols[:], in_=dis2_in[:])
            nc.sync.dma_start(out=msgs1_sb[:, mc0:NT * cpt1, :],
                              in_=msgs1.ap()[:, mc0:NT * cpt1, :])
            dst16 = const.tile([P, NT * cpt], f16)
            nc.sync.dma_start(out=dst16[:], in_=dst_rel[:])
            b2bc = const.tile([P, F2], f32)
            nc.sync.dma_start(out=b2bc[:], in_=b2bc_in[:])
            dis_cols = const.tile([P, NT], f32)
            nc.sync.dma_start(out=dis_cols[:], in_=dis_in[:])
            ident32 = const.tile([P, P], f32)
            nc.sync.dma_start(out=ident32[:], in_=id32_in[:])

            b2g = const.tile([P, GT, F2], f32)
            for j in range(GT):
                nc.vector.tensor_copy(out=b2g[:, j, :], in_=b2bc[:])
            ztf16 = const.tile([P, NT, F2], f16)
            zt8 = const.tile([P, NT - CT[FP8C], 2 * F2], f8)
            pa0 = const.tile([P, NT, F2], f16)
            pa1 = const.tile([P, NT, F2], f16)
            qa = pa1 if (NCH - 2) % 2 == 0 else pa0  # src_pa is dead post-add

            iov = iota16[:].rearrange("p (n c) -> p n c", c=cpt)

            def issue_coll(c):
                if c >= FP8C:
                    nc.gpsimd.collective_compute(
                        "AllGather", mybir.AluOpType.bypass, replica_groups=rg,
                        ins=[ztown8[c - FP8C][:]], outs=[ztg8[c - FP8C][:]],
                    )
                else:
                    nc.gpsimd.collective_compute(
                        "AllGather", mybir.AluOpType.bypass, replica_groups=rg,
                        ins=[ztown[CT[c] * P:CT[c + 1] * P, :]],
                        outs=[ztg[c][:]],
                    )

            # ---------------- pass 1: layer 1 -> zt ----------------
            with (
                tc.tile_pool(name="ps_ga", bufs=3, space="PSUM") as ps_ga,
                tc.tile_pool(name="ps_h1", bufs=3, space="PSUM") as ps_h1,
                tc.tile_pool(name="ps_zt", bufs=2, space="PSUM") as ps_zt,
            ):
                for t in range(NT):
                    # 16-wide one-hot across the tile's 8 eighths, one DVE op
                    oh = ohp.tile([P, H1 * cpt1], f16, tag="oh")
                    ohv = oh[:].rearrange("p (n c) -> p n c", c=cpt1)
                    nc.vector.tensor_tensor(
                        out=ohv[:],
                        in0=dst_l1[:, None, t * cpt1:(t + 1) * cpt1
                                   ].broadcast_to([P, H1, cpt1]),
                        in1=iov[:, 0:H1, 0:cpt1],
                        op=mybir.AluOpType.is_equal,
                    )
                    # feature-major aggregate: ga[f, node] directly (no
                    # transpose stage) — lhsT is the 5-wide message column,
                    # the one-hot streams as rhs
                    ga = ps_ga.tile([FX, P], f32, tag="ga")
                    for o in range(NE1):
                        for k in range(cpt1o):
                            i = o * cpt1o + k
                            nc.tensor.matmul(
                                out=ga[0:FX, o * H1:(o + 1) * H1],
                                lhsT=msgs1_sb[:, t * cpt1 + i, 0:FX],
                                rhs=ohv[:, :, i],
                                start=(k == 0), stop=(k == cpt1o - 1),
                            )
                    s1t = smallp.tile([FX, P], f16, tag="s1t")
                    nc.scalar.copy(out=s1t[:], in_=ga[:])
                    h1p = ps_h1.tile([P, P], f32, tag="h1")
                    nc.tensor.matmul(out=h1p[:], lhsT=w1_sb[:], rhs=s1t[:],
                                     start=True, stop=False)
                    nc.tensor.matmul(out=h1p[:], lhsT=b1row[:],
                                     rhs=invdis_sb[:, t * P:(t + 1) * P],
                                     start=False, stop=True)
                    h1r = smallp.tile([P, P], f16, tag="h1r")
                    nc.scalar.activation(out=h1r[:], in_=h1p[:],
                                         func=mybir.ActivationFunctionType.Relu)
                    ztp = ps_zt.tile([P, F2], f32, tag="ztp")
                    nc.tensor.matmul(out=ztp[:], lhsT=h1r[:], rhs=w2_sb[:],
                                     start=True, stop=True)
                    nc.scalar.activation(out=ztf16[:, t, :], in_=ztp[:],
                                         func=mybir.ActivationFunctionType.Copy,
                                         scale=dis2_cols[:, t:t + 1])
                    if t >= CT[FP8C]:
                        tl8 = t - CT[FP8C]
                        nc.scalar.copy(out=zt8[:, tl8, 0:F2],
                                       in_=ztf16[:, t, :])
                        nc.scalar.copy(out=zt8[:, tl8, F2:2 * F2],
                                       in_=ztf16[:, t, :])
                    if t % ZG == ZG - 1:
                        g = t // ZG
                        if t >= CT[FP8C]:
                            c8 = np.searchsorted(np.array(CT), t, "right") - 1
                            b8 = g * ZG - CT[c8]
                            nc.sync.dma_start(
                                out=ztown8[c8 - FP8C].ap()[
                                    b8 * P:(t + 1 - CT[c8]) * P, :].rearrange(
                                    "(t p) f -> p t f", p=P),
                                in_=zt8[:, g * ZG - CT[FP8C]:t + 1 - CT[FP8C],
                                        :],
                            )
                        else:
                            nc.sync.dma_start(
                                out=ztown.ap()[g * ZG * P:(t + 1) * P,
                                               :].rearrange(
                                    "(t p) f -> p t f", p=P),
                                in_=ztf16[:, g * ZG:t + 1, :],
                            )
                    for c in range(NCH - 1):
                        if t == CT[c + 1] - 1:
                            issue_coll(c)

            # pair-packed views: row r of pv[c] holds nodes (2r, 2r+1) as 256B
            pv = [z.ap().rearrange("(r two) f -> r (two f)", two=2) for z in ztg]
            pv8 = [z.ap().rearrange("(r two) f -> r (two f)", two=2)
                   for z in ztg8]
            dsv = dst16[:].rearrange("p (t i) -> p t i", i=cpt)

            with tc.tile_pool(name="ps_g2", bufs=6, space="PSUM") as ps_g2:
                for c in range(NCH):
                    if c == 1:
                        # issued before phase 1's gathers so it dispatches as
                        # soon as its input is ready (Pool SEQ is in-order)
                        issue_coll(NCH - 1)
                    s0, s1 = scall[2 * c]       # even-src: half0, half1 chunks
                    s2, s3 = scall[2 * c + 1]   # odd-src
                    se, so = s0 + s1, s2 + s3
                    cw = se + so
                    c0 = cbase[4 * c]
                    ie = idxp.tile([P, NT * se * 8], i16, tag="idx0")
                    nc.sync.dma_start(out=ie[:], in_=idx_in[2 * c][:])
                    io = idxp.tile([P, NT * so * 8], i16, tag="idx1")
                    nc.sync.dma_start(out=io[:], in_=idx_in[2 * c + 1][:])

                    def finals(g):
                        ts_ = slice(g * GT, (g + 1) * GT)
                        g2g = g2gs.pop(g)
                        if c == 0:
                            nc.vector.tensor_add(out=pa0[:, ts_, :], in0=g2g[:],
                                                 in1=ztf16[:, ts_, :])
                        elif c < NCH - 1:
                            src_pa = pa0 if c % 2 == 1 else pa1
                            dst_pa = pa1 if c % 2 == 1 else pa0
                            nc.vector.tensor_add(out=dst_pa[:, ts_, :],
                                                 in0=g2g[:],
                                                 in1=src_pa[:, ts_, :])
                            if c == NCH - 2:
                                nc.vector.tensor_tensor(
                                    out=qa[:, ts_, :], in0=dst_pa[:, ts_, :],
                                    in1=dis_cols[:, ts_, None].broadcast_to(
                                        [P, GT, F2]),
                                    op=mybir.AluOpType.mult,
                                )
                                nc.vector.tensor_add(out=qa[:, ts_, :],
                                                     in0=qa[:, ts_, :],
                                                     in1=b2g[:])
                        else:
                            outg = outp.tile([P, GT, F2], f32, tag="outg")
                            nc.vector.tensor_tensor(
                                out=outg[:], in0=g2g[:],
                                in1=dis_cols[:, ts_, None].broadcast_to(
                                    [P, GT, F2]),
                                op=mybir.AluOpType.mult,
                            )
                            nc.vector.tensor_add(out=outg[:], in0=outg[:],
                                                 in1=qa[:, ts_, :])
                            nc.sync.dma_start(
                                out=out_ext.ap()[g * GT * P:(g + 1) * GT * P,
                                                 :].rearrange(
                                    "(t p) f -> p t f", p=P),
                                in_=outg[:],
                            )

                    g2gs = {}
                    UG = 2                 # matmul groups per gather unit
                    for u in range(NGRP // UG):
                        if c >= FP8C:
                            msga = msg2p.tile([P, UG * GT * cw, F2], f8,
                                              tag="msg8")
                            dma_gather_raw(
                                nc.gpsimd, msga[:, 0:UG * GT * se, :],
                                pv8[c - FP8C][:, 0:F2],
                                ie[:, u * UG * GT * se * 8:
                                   (u + 1) * UG * GT * se * 8],
                                UG * GT * se * P, F2, 4 * F2)
                            dma_gather_raw(
                                nc.gpsimd, msga[:, UG * GT * se:UG * GT * cw, :],
                                pv8[c - FP8C][:, 2 * F2:3 * F2],
                                io[:, u * UG * GT * so * 8:
                                   (u + 1) * UG * GT * so * 8],
                                UG * GT * so * P, F2, 4 * F2)
                        else:
                            msga = msg2p.tile([P, UG * GT * cw, F2], f16,
                                              tag="msg2")
                            dma_gather_raw(
                                nc.gpsimd, msga[:, 0:UG * GT * se, :],
                                pv[c][:, 0:F2],
                                ie[:, u * UG * GT * se * 8:
                                   (u + 1) * UG * GT * se * 8],
                                UG * GT * se * P, F2, 2 * F2)
                            dma_gather_raw(
                                nc.gpsimd, msga[:, UG * GT * se:UG * GT * cw, :],
                                pv[c][:, F2:2 * F2],
                                io[:, u * UG * GT * so * 8:
                                   (u + 1) * UG * GT * so * 8],
                                UG * GT * so * P, F2, 2 * F2)
                        for gl in range(UG):
                            g = u * UG + gl
                            # group one-hot, 64-wide: oh[p, n, j, i]
                            oh = ohp2.tile([P, H * GT * cw], f16, tag="oh")
                            ohv = oh[:].rearrange("p (n j i) -> p n j i",
                                                  j=GT, i=cw)
                            nc.vector.tensor_tensor(
                                out=ohv[:],
                                in0=dsv[:, None, g * GT:(g + 1) * GT,
                                        c0:c0 + cw].broadcast_to(
                                    [P, H, GT, cw]),
                                in1=iov[:, 0:H, None, 0:cw].broadcast_to(
                                    [P, H, GT, cw]),
                                op=mybir.AluOpType.is_equal,
                            )
                            g2g = ps_g2.tile([P, GT, F2], f32, tag="g2")
                            g2gs[g] = g2g
                            for j in range(GT):
                                jt = gl * GT + j   # tile within gather unit
                                # columns i: [e-h0 | e-h1 | o-h0 | o-h1]
                                for hh in range(2):
                                    cols = ([(k, jt * se + k)
                                             for k in range(s0)]
                                            if hh == 0 else
                                            [(s0 + k, jt * se + s0 + k)
                                             for k in range(s1)])
                                    ocols = ([(se + k,
                                               UG * GT * se + jt * so + k)
                                              for k in range(s2)]
                                             if hh == 0 else
                                             [(se + s2 + k,
                                               UG * GT * se + jt * so + s2 + k)
                                              for k in range(s3)])
                                    chain = cols + ocols
                                    for q, (i, mcol) in enumerate(chain):
                                        nc.tensor.matmul(
                                            out=g2g[hh * H:(hh + 1) * H, j, :],
                                            lhsT=ohv[:, :, j, i],
                                            rhs=msga[:, mcol, :],
                                            start=(q == 0),
                                            stop=(q == len(chain) - 1),
                                        )
                            if g >= 1:
                                finals(g - 1)
                    finals(NGRP - 1)

    nc.compile()
    return nc


def _wrap_idx(a):
    """Index layout for dma_gather: [j%16, j//16] tiled to 128 partitions."""
    lin = a.reshape(-1)
    w = lin.reshape(-1, 16).T
    return np.ascontiguousarray(np.tile(w, (8, 1)))


def make_inputs(x, edge_index, W1, b1, W2, b2):
    x = np.asarray(x, np.float32)
    ei = np.asarray(edge_index)
    src = ei[0].astype(np.int64)
    dst = ei[1].astype(np.int64)
    E = src.shape[0]

    deg = (np.bincount(dst, minlength=N_NODES) + 1.0).astype(np.float32)
    dis = 1.0 / np.sqrt(deg)
    invdis = np.sqrt(deg)
    dis_pad = np.ones(V, np.float32)
    dis_pad[:N_NODES] = dis
    invdis_pad = np.ones(V, np.float32)
    invdis_pad[:N_NODES] = invdis
    xs16_pad = np.zeros((V, FX), np.float16)
    xs16_pad[:N_NODES] = (x * dis[:, None]).astype(np.float16)

    gtile = (dst >> 7).astype(np.int64)               # 0..390
    dhalf = (dst >> 6) & 1                             # dst half-tile
    r = (src % RANGE).astype(np.int64)
    rt = r >> 7                                        # src's owner-local tile
    chunk = np.searchsorted(np.array(CB, np.int64), rt, side="right")
    cls = (2 * chunk + (src & 1)) * 2 + dhalf
    order = np.lexsort((np.arange(E), cls, gtile))
    s_src = src[order]
    s_dst = dst[order]
    s_gt = gtile[order]
    s_cls = cls[order]

    key = s_gt * NCL + s_cls
    counts = np.bincount(key, minlength=400 * NCL)
    starts = np.zeros(400 * NCL + 1, np.int64)
    np.cumsum(counts, out=starts[1:])
    pos = np.arange(E, dtype=np.int64) - starts[key]

    carr = counts.reshape(400, NCL)
    cpts = [max(1, int(np.ceil(carr[:, c].max() / P))) for c in range(NCL)]
    cpt = sum(cpts)
    cbase = np.zeros(NCL + 1, np.int64)
    np.cumsum(cpts, out=cbase[1:])

    tl = s_gt % NT
    col = tl * cpt + cbase[s_cls] + pos // P
    part = pos % P
    core = s_gt // NT

    # ---- layer 1: append self-loop edges, slot by (tile, eighth) ----
    loops = np.arange(N_NODES, dtype=np.int64)
    l_src = np.concatenate([src, loops])
    l_dst = np.concatenate([dst, loops])
    l_gt = (l_dst >> 7).astype(np.int64)
    l_tl = l_gt % NT
    l_core = l_gt // NT
    kth = l_gt * NE1 + ((l_dst >> 5) & (NE1 - 1))
    counts_th = np.bincount(kth, minlength=400 * NE1)
    starts_th = np.zeros(400 * NE1 + 1, np.int64)
    np.cumsum(counts_th, out=starts_th[1:])
    o2 = np.argsort(kth, kind="stable")
    EL = l_src.shape[0]
    pos_th = np.empty(EL, np.int64)
    pos_th[o2] = np.arange(EL, dtype=np.int64) - starts_th[kth[o2]]
    cpt1o = max(1, int(np.ceil(counts_th.max() / P)))
    cpt1 = NE1 * cpt1o
    col1 = l_tl * cpt1 + ((l_dst >> 5) & (NE1 - 1)) * cpt1o + pos_th // P
    part1 = pos_th % P

    # gather table row (pair index) within the chunk's collective output
    s_r = (s_src % RANGE).astype(np.int64)
    s_ch = s_cls >> 2
    ct0 = np.array(CT[:NCH], np.int64) * P
    rows_c = np.array([P * (CT[c + 1] - CT[c]) for c in range(NCH)], np.int64)
    local = s_r - ct0[s_ch]
    idxval = ((s_src // RANGE) * rows_c[s_ch] + local) >> 1

    w1_16 = np.asarray(W1, np.float16)
    b1row = np.asarray(b1, np.float16).reshape(1, P)
    w2_16 = np.asarray(W2, np.float16)
    b2bc = np.tile(np.asarray(b2, np.float32).reshape(1, F2), (P, 1))
    iota = np.tile(np.repeat(np.arange(H, dtype=np.float16), cpt)[None, :],
                   (P, 1))
    ident32 = np.eye(P, dtype=np.float32)

    scall = [(cpts[4 * c + 2 * rr], cpts[4 * c + 2 * rr + 1])
             for c in range(NCH) for rr in range(2)]

    in_maps = []
    for k in range(NCORES):
        m = core == k
        kc, kp = col[m], part[m]
        ml = l_core == k
        kc1, kp1 = col1[ml], part1[ml]

        msgs1_k = np.zeros((P, NT * cpt1, MW), np.float16)
        msgs1_k[kp1, kc1, 0:FX] = xs16_pad[l_src[ml]]
        dst_l1_k = np.full((P, NT * cpt1), PAD_DST, np.float16)
        dst_l1_k[kp1, kc1] = (l_dst[ml] & (H1 - 1)).astype(np.float16)
        dst16_k = np.full((P, NT * cpt), PAD_DST, np.float16)
        dst16_k[kp, kc] = (s_dst[m] & 63).astype(np.float16)

        idx_maps = {}
        for sidx in range(NCL // 2):
            c, rr = sidx // 2, sidx % 2
            cl0 = 4 * c + 2 * rr        # half-0 class id
            s0, s1 = scall[sidx]
            lin = np.zeros(NT * (s0 + s1) * P, np.int16)
            for hh, (cl, sz, off) in enumerate(
                    (((cl0, s0, 0)), (cl0 + 1, s1, s0))):
                mc = m & (s_cls == cl)
                flat = (tl[mc] * (s0 + s1) + off + pos[mc] // P) * P + part[mc]
                lin[flat] = idxval[mc].astype(np.int16)
            idx_maps[f"idx{sidx}"] = _wrap_idx(lin)

        nsl = slice(k * RANGE, (k + 1) * RANGE)
        dis_k = np.ascontiguousarray(
            dis_pad[nsl].reshape(NT, P).T.astype(np.float32))
        dis2_k = np.ascontiguousarray((dis_k * dis_k).astype(np.float32))
        invdis_k = invdis_pad[nsl].reshape(1, RANGE).astype(np.float16)

        in_maps.append({
            "msgs1": msgs1_k, "dst_l1": dst_l1_k, "dst_rel": dst16_k,
            "iota_in": iota,
            **idx_maps,
            "w1": w1_16, "b1row": b1row, "w2": w2_16,
            "b2bc": b2bc, "invdis": invdis_k,
            "dis_cols": dis_k, "dis2_cols": dis2_k,
            "ident32": ident32,
        })
    return (cpt1o,) + tuple(cpts), in_maps


def kernel(x, edge_index, W1, b1, W2, b2):
    from concourse.bass_utils import run_bass_kernel_spmd

    key, in_maps = make_inputs(x, edge_index, W1, b1, W2, b2)
    if key not in _prog_cache:
        _prog_cache[key] = build_program(*key)
    nc = _prog_cache[key]
    res = run_bass_kernel_spmd(nc, in_maps, list(range(NCORES)))
    out = np.concatenate([res.results[k]["out"] for k in range(NCORES)], axis=0)
    return out[:N_NODES]
